# revision 16
# baseline (speedup 1.0000x reference)
"""MicroTransformer forward pass on 8 trn2 NeuronCores.

Sharding: DP2 (batch) x CP4 (strided context parallel).
Core c = (b, p), b = c // 4, p = c % 4, owns tokens at global positions
p, p+4, p+8, ... of batch b  (T = S/4 tokens per core).

Per layer: k-projection first (K shard dumped + AllGathered in two
feature halves), then v (transposed + ones-augmented, gathered in two
column halves), then q — so the gathers run under projection compute.
Attention per head pair: one wide QK^T matmul per (shard, key-chunk)
into transposed scores [tk, tq]; exp feeds one wide accumulating AV
matmul per (shard, key-chunk).  Out-proj and SwiGLU FFN are
token-local.  LM head: own tokens x full vocab in f32r.  Norm weights
fold into the following weight matrix on the host; weights arrive
pre-transposed and in bf16 so the contraction dim lies on SBUF
partitions.

On-chip layout: activations are feature-major [feat, tok]; matmul
outputs [out_feat, tok] feed the next matmul's moving operand directly.
Partition-dim reductions (RMSNorm sum, softmax denom) use ones-vector /
ones-column matmuls on the PE.
"""

import numpy as np

try:
    import concourse.bass as bass  # noqa: F401
except ImportError:
    import sys

    sys.path.insert(0, "/opt/trn_rl_repo")
    import concourse.bass as bass  # noqa: F401

import ml_dtypes

import concourse.bacc as bacc
import concourse.mybir as mybir
import concourse.tile as tile
from concourse.bass_utils import run_bass_kernel_spmd
from concourse.masks import make_identity

F32 = mybir.dt.float32
F32R = mybir.dt.float32r
BF16 = mybir.dt.bfloat16
AF = mybir.ActivationFunctionType

NEG = -1e30

CFG_FULL = dict(V=32000, D=1024, L=8, F=4096, S=2048, H=16, HD=64, LM_OC=5)


class Ctx:
    def __init__(self, cfg):
        self.__dict__.update(cfg)
        self.B = 2
        self.T = self.S // 4        # tokens per core
        self.DT = self.D // 128     # 128-row feature tiles of x/h
        self.FT = self.F // 128
        self.NTK = self.T // 128    # tk chunks per shard (= tq blocks)
        assert self.T % 128 == 0 and self.D % 128 == 0 and self.HD == 64
        self.VW = (self.H // 2) * 132  # per pair: ones|v_even|ones|v_odd|pp
        self.VWH = self.VW // 2
        self.KH = (self.D // 2) * self.T       # K elems per feature half
        self.VH = self.T * self.VWH            # V elems per column half


def _t(pool, shape, dtype, tag):
    return pool.tile(shape, dtype, tag=tag, name=tag)


def _r(ap):
    return ap.bitcast(F32R)


def build_nc(cfg, n_cores=8):
    c = Ctx(cfg)
    nc = bacc.Bacc("TRN2", target_bir_lowering=False, debug=False,
                   num_devices=n_cores)
    if n_cores == 8:
        groups = [[0, 1, 2, 3], [4, 5, 6, 7]]
    else:
        groups = [list(range(n_cores))]

    D, T, L, F, V = c.D, c.T, c.L, c.F, c.V

    io = {}
    def inp(name, shape, dt=F32):
        io[name] = nc.dram_tensor(name, shape, dt, kind="ExternalInput").ap()
    inp("x0", [D, T])
    inp("qkvT", [L, D, 3 * D], BF16)
    inp("owT", [L, D, D], BF16)
    inp("w1T", [L, D, F], BF16)
    inp("w3T", [L, D, F], BF16)
    inp("w2T", [L, F, D], BF16)
    inp("embT", [D, V])
    inp("cosq", [128, T], BF16)
    inp("sinq", [128, T], BF16)
    inp("ropeP", [128, 128], BF16)
    inp("maskA", [4, 128, 128])
    io["logits"] = nc.dram_tensor("logits", [V, T], F32,
                                  kind="ExternalOutput").ap()

    with tile.TileContext(nc) as tc:
        _emit(tc, c, groups, io)
    nc.compile()
    return nc


def _emit(tc, c, groups, io):
    nc = tc.nc
    D, T, L, F, V, H = c.D, c.T, c.L, c.F, c.V, c.H
    DT, FT, NTK, VW = c.DT, c.FT, c.NTK, c.VW
    FQ = 4 if FT % 4 == 0 else FT
    scale = c.HD ** -0.5
    WCOL = 512

    ctx_pools = []
    def pool(**kw):
        p = tc.tile_pool(**kw)
        v = p.__enter__()
        ctx_pools.append(p)
        return v

    perst = pool(name="perst", bufs=1)
    wpool = pool(name="wpool", bufs=2)      # streamed weight tiles [128, 512]
    apool = pool(name="apool", bufs=1)      # per-layer activations (by tag)
    spool1 = pool(name="spool1", bufs=1)    # norm staging
    spool2 = pool(name="spool2", bufs=2)    # rotating staging tiles
    kpool = pool(name="kpool", bufs=2)      # gathered K tiles, per shard
    vgpool = pool(name="vgpool", bufs=2)    # gathered V tiles, per shard
    epool = pool(name="epool", bufs=8)      # exp tiles
    gpool = pool(name="gpool", bufs=1)      # gate tiles
    gupool = pool(name="gupool", bufs=1)    # gate*up tiles
    ps = pool(name="ps", bufs=2, space="PSUM")
    ps_s = pool(name="ps_s", bufs=4, space="PSUM")
    ps_o = pool(name="ps_o", bufs=2, space="PSUM")
    dram = pool(name="dram", bufs=2, space="DRAM")

    # ---------- persistent tiles ----------
    xt = [_t(perst, [128, T], F32, f"x{i}") for i in range(DT)]
    cos_t = _t(perst, [128, T], BF16, "cos")
    sin_t = _t(perst, [128, T], BF16, "sin")
    ropeP_t = _t(perst, [128, 128], BF16, "ropeP")
    ident_t = _t(perst, [128, 128], F32, "ident")
    identb_t = _t(perst, [128, 128], BF16, "identb")
    mask_t = [_t(perst, [128, 128], F32, f"mask{s}") for s in range(4)]
    ones_t = _t(perst, [128, 1], F32, "ones")
    eps_t = _t(perst, [1, 1], F32, "eps")
    nc.gpsimd.memset(eps_t[:], 1e-6)

    nc.sync.dma_start(cos_t[:], io["cosq"][:])
    nc.sync.dma_start(sin_t[:], io["sinq"][:])
    nc.sync.dma_start(ropeP_t[:], io["ropeP"][:])
    for s in range(4):
        nc.sync.dma_start(mask_t[s][:], io["maskA"][s])
    ones_raw = _t(perst, [128, 1], F32, "ones_raw")
    nc.gpsimd.memset(ones_raw[:], 1.0)
    nc.vector.tensor_copy(_r(ones_t[:]), ones_raw[:])
    make_identity(nc, ident_t[:])
    nc.vector.tensor_copy(identb_t[:], ident_t[:])
    for i in range(DT):
        nc.sync.dma_start(xt[i][:], io["x0"][i * 128:(i + 1) * 128, :])

    def load_w(dram_ap, r0, c0, rows=128, cols=WCOL, dt=BF16, tagp="w"):
        t = _t(wpool, [128, WCOL], dt, f"{tagp}{(r0 // 128) % 8}")
        if dt == F32:
            nc.sync.dma_start(_r(t[:rows, :cols]),
                              _r(dram_ap[r0:r0 + rows, c0:c0 + cols]))
        else:
            nc.sync.dma_start(t[:rows, :cols],
                              dram_ap[r0:r0 + rows, c0:c0 + cols])
        return t

    def rmsnorm(out_dt):
        """h = x * rsqrt(mean(x^2) + eps). Returns h tiles (feature-major)."""
        ssum = _t(ps, [128, T], F32, "mm")
        for i in range(DT):
            sqt = _t(spool1, [128, T], F32, "nsq")
            nc.vector.tensor_mul(_r(sqt[:]), xt[i][:], xt[i][:])
            nc.tensor.matmul(ssum[0:1, :], _r(ones_t[:]), _r(sqt[:]),
                             start=(i == 0), stop=(i == DT - 1))
        srt = _t(spool1, [1, T], F32, "nsrt")
        nc.scalar.activation(srt[:], ssum[0:1, :], AF.Sqrt,
                             bias=eps_t[:], scale=1.0 / D)
        sb = _t(spool1, [128, T], F32, "nsb")
        nc.gpsimd.partition_broadcast(sb[:], srt[:])
        rb = _t(spool1, [128, T], F32, "nrb")
        nc.vector.reciprocal(rb[:], sb[:])
        hts = []
        for i in range(DT):
            h = _t(apool, [128, T], out_dt, f"h{i}")
            dst = _r(h[:]) if out_dt == F32 else h[:]
            nc.vector.tensor_mul(dst, xt[i][:], rb[:])
            hts.append(h)
        return hts

    def proj_section(layer, which, hts):
        """Compute one D-section of qkv: returns list of PSUM tiles consumed
        by caller one at a time (generator to keep PSUM pressure low)."""
        wsec = []
        for k in range(DT):
            wt0 = load_w(io["qkvT"][layer], k * 128, which * D,
                         cols=min(WCOL, D))
            wt1 = load_w(io["qkvT"][layer], k * 128, which * D + WCOL,
                         tagp="wb") if D > WCOL else None
            wsec.append((wt0, wt1))
        for i in range(DT):
            pm = _t(ps, [128, T], F32, "mm")
            col = i * 128
            for k in range(DT):
                wt = wsec[k][0] if col < WCOL else wsec[k][1]
                cc = col % WCOL
                nc.tensor.matmul(pm[:], wt[:, cc:cc + 128], hts[k][:],
                                 start=(k == 0), stop=(k == DT - 1))
            yield i, pm

    def rope(pm, out, i):
        """out = pm*cos + rotate_half(pm)*sin   (bf16 out)."""
        sb = _t(spool2, [128, T], BF16, "rsb")
        nc.vector.tensor_copy(sb[:], pm[:])
        rot = _t(ps, [128, T], F32, "mm")
        nc.tensor.matmul(rot[:], ropeP_t[:], sb[:], start=True, stop=True)
        t1 = _t(spool2, [128, T], BF16, "rope1")
        nc.vector.tensor_mul(t1[:], sb[:], cos_t[:])
        t2 = _t(spool2, [128, T], BF16, "rope2")
        nc.vector.tensor_mul(t2[:], rot[:], sin_t[:])
        nc.vector.tensor_add(out[:], t1[:], t2[:])

    for layer in range(L):
        # ================= attention =================
        hts = rmsnorm(BF16)

        qp = [_t(apool, [128, T], BF16, f"qp{i}") for i in range(DT)]
        vT = [_t(apool, [128, VW], BF16, f"vT{b}") for b in range(NTK)]
        for b in range(NTK):
            nc.gpsimd.memset(vT[b][:], 1.0)

        ksh = [dram.tile([c.KH], BF16, tag=f"ksh{h}", name=f"ksh{h}")
               for h in range(2)]
        kall = [dram.tile([4 * c.KH], BF16, tag=f"kall{h}", name=f"kall{h}")
                for h in range(2)]
        vsh = [dram.tile([c.VH], BF16, tag=f"vsh{h}", name=f"vsh{h}")
               for h in range(2)]
        vall = [dram.tile([4 * c.VH], BF16, tag=f"vall{h}", name=f"vall{h}")
                for h in range(2)]

        # --- k section: project + RoPE, dump, gather per feature half ---
        for i, pm in proj_section(layer, 1, hts):
            kb = _t(spool2, [128, T], BF16, "kb")
            rope(pm, kb, i)
            hh = i // (DT // 2)
            off = (i % (DT // 2)) * 128 * T
            nc.sync.dma_start(
                ksh[hh][off:off + 128 * T].rearrange("(p t) -> p t", p=128),
                kb[:])
            if i % (DT // 2) == DT // 2 - 1:
                nc.gpsimd.collective_compute(
                    "AllGather", mybir.AluOpType.bypass,
                    replica_groups=groups,
                    ins=[ksh[hh].opt()], outs=[kall[hh].opt()])

        # --- v section: project, transpose into augmented layout, dump ---
        for i, pm in proj_section(layer, 2, hts):
            vsb = _t(spool2, [128, T], BF16, "vsb")
            nc.vector.tensor_copy(vsb[:], pm[:])
            for b in range(NTK):
                pt = _t(ps_s, [128, 128], BF16, "st")
                nc.tensor.transpose(pt[:], vsb[:, b * 128:(b + 1) * 128],
                                    identb_t[:])
                nc.vector.tensor_copy(vT[b][:, 132 * i:132 * i + 64],
                                      pt[:, 0:64])
                nc.vector.tensor_copy(vT[b][:, 132 * i + 65:132 * i + 129],
                                      pt[:, 64:128])
            if i == DT // 2 - 1 or i == DT - 1:
                hh = i // (DT // 2)
                c0 = hh * c.VWH
                for b in range(NTK):
                    off = b * 128 * c.VWH
                    nc.sync.dma_start(
                        vsh[hh][off:off + 128 * c.VWH].rearrange(
                            "(p t) -> p t", p=128),
                        vT[b][:, c0:c0 + c.VWH])
                nc.gpsimd.collective_compute(
                    "AllGather", mybir.AluOpType.bypass,
                    replica_groups=groups,
                    ins=[vsh[hh].opt()], outs=[vall[hh].opt()])

        # --- q section ---
        for i, pm in proj_section(layer, 0, hts):
            rope(pm, qp[i], i)

        # --- attention: head pair i uses K feature tile i, V col block i ---
        for i in range(DT):
            kh = i // (DT // 2)
            kg = []
            for s in range(4):
                t = _t(kpool, [128, T], BF16, f"kg{s}")
                off = s * c.KH + (i % (DT // 2)) * 128 * T
                nc.sync.dma_start(
                    t[:], kall[kh][off:off + 128 * T].rearrange(
                        "(p t) -> p t", p=128))
                kg.append(t)
            vh = i // (DT // 2)
            ic = (i % (DT // 2)) * 132
            vg = []
            for s in range(4):
                t = _t(vgpool, [128, NTK * 132], BF16, f"vg{s}")
                src = vall[vh][s * c.VH:(s + 1) * c.VH].rearrange(
                    "(b p t) -> p b t", p=128, t=c.VWH)
                nc.sync.dma_start(
                    t[:].rearrange("p (b t) -> p b t", t=132),
                    src[:, :, ic:ic + 132])
                vg.append(t)

            aop = _t(spool2, [128, T], BF16, "aop")
            blocks = [(ck, s) for ck in range(NTK) for s in range(4)]
            NB = len(blocks)
            for hh in range(2):
                r0 = hh * 64
                o_ps = _t(ps_o, [128, T], F32, "oaug")
                elist = [None] * NB

                def emit_qk(n):
                    ck, s = blocks[n]
                    q0 = ck * 128
                    st = _t(ps_s, [128, T], F32, "st")
                    nc.tensor.matmul(st[:, q0:T],
                                     kg[s][r0:r0 + 64, q0:q0 + 128],
                                     qp[i][r0:r0 + 64, q0:T],
                                     start=True, stop=True)
                    nc.vector.tensor_add(st[:, q0:q0 + 128],
                                         st[:, q0:q0 + 128], mask_t[s][:])
                    e = _t(epool, [128, T], BF16, "e")
                    nc.scalar.activation(e[:, q0:T], st[:, q0:T], AF.Exp,
                                         scale=scale)
                    elist[n] = e

                def emit_av(n):
                    ck, s = blocks[n]
                    q0 = ck * 128
                    e = elist[n]
                    # [v_half | ones]: rows 0:64 = o, row 64 = den
                    c0 = 132 * ck + 65 * hh
                    nc.tensor.matmul(
                        o_ps[0:65, q0:T],
                        vg[s][:, c0:c0 + 65],
                        e[:, q0:T],
                        start=(n == 0), stop=(n == NB - 1),
                        skip_group_check=True)

                LEAD = 4
                for n in range(min(LEAD, NB)):
                    emit_qk(n)
                for n in range(NB):
                    if n + LEAD < NB:
                        emit_qk(n + LEAD)
                    emit_av(n)
                den = _t(spool2, [1, T], F32, "den")
                nc.vector.tensor_copy(den[:], o_ps[64:65, :])
                denb = _t(spool2, [128, T], F32, "denb")
                nc.gpsimd.partition_broadcast(denb[:], den[:])
                rb128 = _t(spool2, [128, T], F32, "recb")
                nc.vector.reciprocal(rb128[:], denb[:])
                nc.vector.tensor_mul(aop[r0:r0 + 64, :], o_ps[0:64, :],
                                     rb128[r0:r0 + 64, :])
            # out-projection contribution of this head pair, into residual
            wo0 = load_w(io["owT"][layer], i * 128, 0, cols=min(WCOL, D))
            wo1 = load_w(io["owT"][layer], i * 128, WCOL, tagp="wb") \
                if D > WCOL else None
            for oc in range(DT):
                pm = _t(ps, [128, T], F32, "mm")
                col = oc * 128
                wt = wo0 if col < WCOL else wo1
                cc = col % WCOL
                nc.tensor.matmul(pm[:], wt[:, cc:cc + 128], aop[:],
                                 start=True, stop=True)
                nc.vector.tensor_add(xt[oc][:], xt[oc][:], pm[:])

        # ================= FFN (SwiGLU), quarter-fused =================
        hts = rmsnorm(BF16)
        for q0f in range(0, FT, FQ):
            f1 = min(q0f + FQ, FT)
            nf = f1 - q0f
            w1 = [load_w(io["w1T"][layer], k * 128, q0f * 128,
                         cols=min(WCOL, nf * 128)) for k in range(DT)]
            g = []
            for f in range(q0f, f1):
                pm = _t(ps, [128, T], F32, "mm")
                for k in range(DT):
                    cc = (f - q0f) * 128
                    nc.tensor.matmul(pm[:], w1[k][:, cc:cc + 128], hts[k][:],
                                     start=(k == 0), stop=(k == DT - 1))
                sg = _t(spool2, [128, T], F32, "sg")
                nc.scalar.activation(sg[:], pm[:], AF.Sigmoid)
                gt = _t(gpool, [128, T], F32, f"g{f - q0f}")
                nc.vector.tensor_mul(gt[:], sg[:], pm[:])
                g.append(gt)
            w3 = [load_w(io["w3T"][layer], k * 128, q0f * 128,
                         cols=min(WCOL, nf * 128), tagp="wb")
                  for k in range(DT)]
            gu = []
            for f in range(q0f, f1):
                pm = _t(ps, [128, T], F32, "mm")
                for k in range(DT):
                    cc = (f - q0f) * 128
                    nc.tensor.matmul(pm[:], w3[k][:, cc:cc + 128], hts[k][:],
                                     start=(k == 0), stop=(k == DT - 1))
                gut = _t(gupool, [128, T], BF16, f"gu{f - q0f}")
                nc.vector.tensor_mul(gut[:], g[f - q0f][:], pm[:])
                gu.append(gut)
            w2 = []
            for f in range(q0f, f1):
                wt0 = load_w(io["w2T"][layer], f * 128, 0, cols=min(WCOL, D))
                wt1 = load_w(io["w2T"][layer], f * 128, WCOL, tagp="wb") \
                    if D > WCOL else None
                w2.append((wt0, wt1))
            for oc in range(DT):
                pm = _t(ps, [128, T], F32, "mm")
                col = oc * 128
                for f in range(nf):
                    wt = w2[f][0] if col < WCOL else w2[f][1]
                    cc = col % WCOL
                    nc.tensor.matmul(pm[:], wt[:, cc:cc + 128], gu[f][:],
                                     start=(f == 0), stop=(f == nf - 1))
                nc.vector.tensor_add(xt[oc][:], xt[oc][:], pm[:])

    # ================= final norm + LM head =================
    hts = rmsnorm(F32)
    VB = c.LM_OC * 128
    assert V % VB == 0
    for vb in range(V // VB):
        we = [load_w(io["embT"], k * 128, vb * VB, cols=min(WCOL, VB), dt=F32)
              for k in range(DT)]
        we2 = None
        if VB > WCOL:
            we2 = [_t(wpool, [128, WCOL], F32, f"wb{k % 8}")
                   for k in range(DT)]
            for k in range(DT):
                nc.sync.dma_start(
                    _r(we2[k][:, :VB - WCOL]),
                    _r(io["embT"][k * 128:(k + 1) * 128,
                                  vb * VB + WCOL:(vb + 1) * VB]))
        for o in range(c.LM_OC):
            pm = _t(ps, [128, T], F32, "mm")
            col = o * 128
            for k in range(DT):
                wt = we[k] if col < WCOL else we2[k]
                cc = col % WCOL
                nc.tensor.matmul(pm[:], _r(wt[:, cc:cc + 128]), _r(hts[k][:]),
                                 start=(k == 0), stop=(k == DT - 1))
            sb = _t(spool2, [128, T], F32, "lmo")
            nc.scalar.copy(sb[:], pm[:])
            nc.sync.dma_start(
                io["logits"][vb * VB + col: vb * VB + col + 128, :], sb[:])

    for p in reversed(ctx_pools):
        p.__exit__(None, None, None)


# ============================================================
# host side
# ============================================================

def host_prep(cfg, inputs, n_cores=8):
    c = Ctx(cfg)
    ids = np.asarray(inputs["input_ids"])
    emb = np.asarray(inputs["emb"], np.float32)
    anw = np.asarray(inputs["attn_norm_w"], np.float32)
    fnw = np.asarray(inputs["ffn_norm_w"], np.float32)
    lnw = np.asarray(inputs["final_norm_w"], np.float32)
    bf = ml_dtypes.bfloat16

    qkvT = np.ascontiguousarray(
        (np.transpose(np.asarray(inputs["qkv_w"], np.float32), (0, 2, 1))
         * anw[:, :, None]).astype(bf))
    owT = np.ascontiguousarray(
        np.transpose(np.asarray(inputs["out_w"], np.float32),
                     (0, 2, 1)).astype(bf))
    w1T = np.ascontiguousarray(
        (np.transpose(np.asarray(inputs["w1"], np.float32), (0, 2, 1))
         * fnw[:, :, None]).astype(bf))
    w3T = np.ascontiguousarray(
        (np.transpose(np.asarray(inputs["w3"], np.float32), (0, 2, 1))
         * fnw[:, :, None]).astype(bf))
    w2T = np.ascontiguousarray(
        np.transpose(np.asarray(inputs["w2"], np.float32),
                     (0, 2, 1)).astype(bf))
    embT = np.ascontiguousarray(emb.T * lnw[:, None])

    hd = c.HD
    inv = 1.0 / (10000.0 ** (np.arange(0, hd, 2, dtype=np.float32) / hd))

    P = np.zeros((128, 128), np.float32)
    for head in range(2):
        b = head * 64
        for m in range(32):
            P[b + m + 32, b + m] = -1.0   # rot[j] = -q[j+32], j < 32
            P[b + m, b + m + 32] = 1.0    # rot[j] =  q[j-32], j >= 32
    k_idx = np.arange(128)[:, None]
    j_idx = np.arange(128)[None, :]
    tri_incl = np.where(k_idx <= j_idx, 0.0, NEG).astype(np.float32)
    tri_strict = np.where(k_idx < j_idx, 0.0, NEG).astype(np.float32)

    in_maps = []
    for core in range(n_cores):
        b, p = core // 4, core % 4
        tok = np.asarray(ids[b, p::4], np.int64)
        x0 = np.ascontiguousarray(emb[tok].T)
        pos = np.arange(p, c.S, 4, dtype=np.float32)
        fr = pos[:, None] * inv[None, :]
        ang = np.concatenate([fr, fr], axis=1)          # [T, hd]
        cosq = np.ascontiguousarray(np.tile(np.cos(ang).T, (2, 1)).astype(bf))
        sinq = np.ascontiguousarray(np.tile(np.sin(ang).T, (2, 1)).astype(bf))
        maskA = np.ascontiguousarray(
            np.stack([tri_incl if s <= p else tri_strict for s in range(4)]))
        in_maps.append(dict(
            x0=x0, qkvT=qkvT, owT=owT, w1T=w1T, w3T=w3T, w2T=w2T, embT=embT,
            cosq=cosq, sinq=sinq, ropeP=P.astype(bf), maskA=maskA))
    return in_maps


def assemble(cfg, results):
    c = Ctx(cfg)
    out = np.empty((c.B, c.S, c.V), np.float32)
    for core in range(len(results)):
        b, p = core // 4, core % 4
        out[b, p::4, :] = results[core]["logits"].T
    return out


_NC_CACHE = {}


def kernel(**inputs):
    cfg = CFG_FULL
    if "full" not in _NC_CACHE:
        _NC_CACHE["full"] = build_nc(cfg)
    nc = _NC_CACHE["full"]
    in_maps = host_prep(cfg, inputs)
    res = run_bass_kernel_spmd(nc, in_maps, list(range(8)))
    return assemble(cfg, res.results)


# revision 27
# speedup vs baseline: 1.2536x; 1.2536x over previous
"""MicroTransformer forward pass on 8 trn2 NeuronCores.

Sharding: DP2 (batch) x CP4 (strided context parallel).
Core c = (b, p), b = c // 4, p = c % 4, owns tokens at global positions
p, p+4, p+8, ... of batch b  (T = S/4 tokens per core).

Per layer: k-projection first (K shard dumped + AllGathered in two
feature halves), then v (transposed + ones-augmented, gathered in two
column halves), then q — so the gathers run under projection compute.
Attention per head pair: one wide QK^T matmul per (shard, key-chunk)
into transposed scores [tk, tq]; exp feeds one wide accumulating AV
matmul per (shard, key-chunk).  Out-proj and SwiGLU FFN are
token-local.  LM head: own tokens x full vocab in f32r.  Norm weights
fold into the following weight matrix on the host; weights arrive
pre-transposed and in bf16 so the contraction dim lies on SBUF
partitions.

On-chip layout: activations are feature-major [feat, tok]; matmul
outputs [out_feat, tok] feed the next matmul's moving operand directly.
Partition-dim reductions (RMSNorm sum, softmax denom) use ones-vector /
ones-column matmuls on the PE.
"""

import numpy as np

try:
    import concourse.bass as bass  # noqa: F401
except ImportError:
    import sys

    sys.path.insert(0, "/opt/trn_rl_repo")
    import concourse.bass as bass  # noqa: F401

import ml_dtypes

import concourse.bacc as bacc
import concourse.mybir as mybir
import concourse.tile as tile
import concourse.bass_utils as _bu
from concourse.bass_utils import run_bass_kernel_spmd
from concourse.masks import make_identity

# (LDW optimization was tried here and rejected: walrus hard-errors with
# "InstLdweights is not compatible with LDW optimization" on transposes.)

F32 = mybir.dt.float32
F32R = mybir.dt.float32r
BF16 = mybir.dt.bfloat16
AF = mybir.ActivationFunctionType

NEG = -1e30

CFG_FULL = dict(V=32000, D=1024, L=8, F=4096, S=2048, H=16, HD=64, LM_OC=5)


class Ctx:
    def __init__(self, cfg):
        self.__dict__.update(cfg)
        self.B = 2
        self.T = self.S // 4        # tokens per core
        self.DT = self.D // 128     # 128-row feature tiles of x/h
        self.FT = self.F // 128
        self.NTK = self.T // 128    # tk chunks per shard (= tq blocks)
        assert self.T % 128 == 0 and self.D % 128 == 0 and self.HD == 64
        self.VW = (self.H // 2) * 132  # per pair: ones|v_even|ones|v_odd|pp
        self.VWH = self.VW // 2
        self.KH = (self.D // 2) * self.T       # K elems per feature half
        self.VH = self.T * self.VWH            # V elems per column half


def _t(pool, shape, dtype, tag):
    return pool.tile(shape, dtype, tag=tag, name=tag)


def _r(ap):
    return ap.bitcast(F32R)


def build_nc(cfg, n_cores=8):
    c = Ctx(cfg)
    nc = bacc.Bacc("TRN2", target_bir_lowering=False, debug=False,
                   num_devices=n_cores)
    if n_cores == 8:
        groups = [[0, 1, 2, 3], [4, 5, 6, 7]]
    else:
        groups = [list(range(n_cores))]

    D, T, L, F, V = c.D, c.T, c.L, c.F, c.V

    io = {}
    def inp(name, shape, dt=F32):
        io[name] = nc.dram_tensor(name, shape, dt, kind="ExternalInput").ap()
    inp("x0", [D, T])
    inp("qkvT", [L, D, 3 * D], BF16)
    inp("owT", [L, D, D], BF16)
    inp("w1T", [L, D, F], BF16)
    inp("w3T", [L, D, F], BF16)
    inp("w2T", [L, F, D], BF16)
    inp("embT", [D, V])
    inp("cosq", [128, T], BF16)
    inp("sinq", [128, T], BF16)
    inp("ropeP", [128, 128], BF16)
    inp("maskA", [4, 128, 128], BF16)
    io["logits"] = nc.dram_tensor("logits", [V, T], F32,
                                  kind="ExternalOutput").ap()

    with tile.TileContext(nc) as tc:
        _emit(tc, c, groups, io)
    nc.compile()
    return nc


def _emit(tc, c, groups, io):
    nc = tc.nc
    D, T, L, F, V, H = c.D, c.T, c.L, c.F, c.V, c.H
    DT, FT, NTK, VW = c.DT, c.FT, c.NTK, c.VW
    FQ = 4 if FT % 4 == 0 else FT
    scale = c.HD ** -0.5
    WCOL = 512

    ctx_pools = []
    def pool(**kw):
        p = tc.tile_pool(**kw)
        v = p.__enter__()
        ctx_pools.append(p)
        return v

    perst = pool(name="perst", bufs=1)
    wpool = pool(name="wpool", bufs=2)      # streamed weight tiles [128, 512]
    apool = pool(name="apool", bufs=1)      # per-layer activations (by tag)
    spool1 = pool(name="spool1", bufs=1)    # norm staging
    spool2 = pool(name="spool2", bufs=2)    # rotating staging tiles
    kpool = pool(name="kpool", bufs=2)      # gathered K tiles, per shard
    vgpool = pool(name="vgpool", bufs=2)    # gathered V tiles, per shard
    epool = pool(name="epool", bufs=8)      # exp tiles
    gpool = pool(name="gpool", bufs=1)      # gate tiles
    gupool = pool(name="gupool", bufs=1)    # gate*up tiles
    ps = pool(name="ps", bufs=2, space="PSUM")
    ps_s = pool(name="ps_s", bufs=4, space="PSUM")
    ps_o = pool(name="ps_o", bufs=2, space="PSUM")
    dram = pool(name="dram", bufs=2, space="DRAM")

    # ---------- persistent tiles ----------
    xt = [_t(perst, [128, T], F32, f"x{i}") for i in range(DT)]
    cos_t = _t(perst, [128, T], BF16, "cos")
    sin_t = _t(perst, [128, T], BF16, "sin")
    ropeP_t = _t(perst, [128, 128], BF16, "ropeP")
    ident_t = _t(perst, [128, 128], F32, "ident")
    identb_t = _t(perst, [128, 128], BF16, "identb")
    mask_t = [_t(perst, [128, 128], BF16, f"mask{s}") for s in range(4)]
    ones_t = _t(perst, [128, 1], F32, "ones")
    eps_t = _t(perst, [1, 1], F32, "eps")
    nc.gpsimd.memset(eps_t[:], 1e-6)

    nc.sync.dma_start(cos_t[:], io["cosq"][:])
    nc.sync.dma_start(sin_t[:], io["sinq"][:])
    nc.sync.dma_start(ropeP_t[:], io["ropeP"][:])
    for s in range(4):
        nc.sync.dma_start(mask_t[s][:], io["maskA"][s])
    ones_raw = _t(perst, [128, 1], F32, "ones_raw")
    nc.gpsimd.memset(ones_raw[:], 1.0)
    nc.vector.tensor_copy(_r(ones_t[:]), ones_raw[:])
    make_identity(nc, ident_t[:])
    nc.vector.tensor_copy(identb_t[:], ident_t[:])
    for i in range(DT):
        nc.sync.dma_start(xt[i][:], io["x0"][i * 128:(i + 1) * 128, :])

    def load_w(dram_ap, r0, c0, rows=128, cols=WCOL, dt=BF16, tagp="w"):
        t = _t(wpool, [128, WCOL], dt, f"{tagp}{(r0 // 128) % 8}")
        if dt == F32:
            nc.sync.dma_start(_r(t[:rows, :cols]),
                              _r(dram_ap[r0:r0 + rows, c0:c0 + cols]))
        else:
            nc.sync.dma_start(t[:rows, :cols],
                              dram_ap[r0:r0 + rows, c0:c0 + cols])
        return t

    def rmsnorm(out_dt):
        """h = x * rsqrt(mean(x^2) + eps). Returns h tiles (feature-major)."""
        ssum = _t(ps, [128, T], F32, "mm")
        for i in range(DT):
            sqt = _t(spool1, [128, T], F32, "nsq")
            nc.vector.tensor_mul(_r(sqt[:]), xt[i][:], xt[i][:])
            nc.tensor.matmul(ssum[0:1, :], _r(ones_t[:]), _r(sqt[:]),
                             start=(i == 0), stop=(i == DT - 1))
        srt = _t(spool1, [1, T], F32, "nsrt")
        nc.scalar.activation(srt[:], ssum[0:1, :], AF.Sqrt,
                             bias=eps_t[:], scale=1.0 / D)
        sb = _t(spool1, [128, T], F32, "nsb")
        nc.gpsimd.partition_broadcast(sb[:], srt[:])
        rb = _t(spool1, [128, T], F32, "nrb")
        nc.vector.reciprocal_approx_fast(rb[:], sb[:])
        hts = []
        for i in range(DT):
            h = _t(apool, [128, T], out_dt, f"h{i}")
            dst = _r(h[:]) if out_dt == F32 else h[:]
            nc.vector.tensor_mul(dst, xt[i][:], rb[:])
            hts.append(h)
        return hts

    def proj_section(layer, which, hts):
        """Compute one D-section of qkv: returns list of PSUM tiles consumed
        by caller one at a time (generator to keep PSUM pressure low)."""
        wsec = []
        for k in range(DT):
            wt0 = load_w(io["qkvT"][layer], k * 128, which * D,
                         cols=min(WCOL, D))
            wt1 = load_w(io["qkvT"][layer], k * 128, which * D + WCOL,
                         tagp="wb") if D > WCOL else None
            wsec.append((wt0, wt1))
        for i in range(DT):
            pm = _t(ps, [128, T], F32, "mm")
            col = i * 128
            for k in range(DT):
                wt = wsec[k][0] if col < WCOL else wsec[k][1]
                cc = col % WCOL
                nc.tensor.matmul(pm[:], wt[:, cc:cc + 128], hts[k][:],
                                 start=(k == 0), stop=(k == DT - 1))
            yield i, pm

    def rope(pm, out, i):
        """out = pm*cos + rotate_half(pm)*sin   (bf16 out)."""
        sb = _t(spool2, [128, T], BF16, "rsb")
        nc.vector.tensor_copy(sb[:], pm[:])
        rot = _t(ps, [128, T], F32, "mm")
        nc.tensor.matmul(rot[:], ropeP_t[:], sb[:], start=True, stop=True)
        t1 = _t(spool2, [128, T], BF16, "rope1")
        nc.vector.tensor_mul(t1[:], sb[:], cos_t[:])
        t2 = _t(spool2, [128, T], BF16, "rope2")
        nc.vector.tensor_mul(t2[:], rot[:], sin_t[:])
        nc.vector.tensor_add(out[:], t1[:], t2[:])

    for layer in range(L):
        # ================= attention =================
        hts = rmsnorm(BF16)

        qp = [_t(apool, [128, T], BF16, f"qp{i}") for i in range(DT)]
        vT = [_t(apool, [128, VW], BF16, f"vT{b}") for b in range(NTK)]
        for b in range(NTK):
            nc.gpsimd.memset(vT[b][:], 1.0)

        ksh = [dram.tile([c.KH], BF16, tag=f"ksh{h}", name=f"ksh{h}")
               for h in range(2)]
        kall = [dram.tile([4 * c.KH], BF16, tag=f"kall{h}", name=f"kall{h}")
                for h in range(2)]
        vsh = [dram.tile([c.VH], BF16, tag=f"vsh{h}", name=f"vsh{h}")
               for h in range(2)]
        vall = [dram.tile([4 * c.VH], BF16, tag=f"vall{h}", name=f"vall{h}")
                for h in range(2)]

        # --- k section: project + RoPE, dump, gather per feature half ---
        for i, pm in proj_section(layer, 1, hts):
            kb = _t(spool2, [128, T], BF16, "kb")
            rope(pm, kb, i)
            hh = i // (DT // 2)
            off = (i % (DT // 2)) * 128 * T
            nc.sync.dma_start(
                ksh[hh][off:off + 128 * T].rearrange("(p t) -> p t", p=128),
                kb[:])
            if i % (DT // 2) == DT // 2 - 1:
                nc.gpsimd.collective_compute(
                    "AllGather", mybir.AluOpType.bypass,
                    replica_groups=groups,
                    ins=[ksh[hh].opt()], outs=[kall[hh].opt()])

        # --- v section: project, transpose into augmented layout, dump ---
        for i, pm in proj_section(layer, 2, hts):
            vsb = _t(spool2, [128, T], BF16, "vsb")
            nc.vector.tensor_copy(vsb[:], pm[:])
            for b in range(NTK):
                pt = _t(ps_s, [128, 128], BF16, "st")
                nc.tensor.transpose(pt[:], vsb[:, b * 128:(b + 1) * 128],
                                    identb_t[:])
                nc.vector.tensor_copy(vT[b][:, 132 * i:132 * i + 64],
                                      pt[:, 0:64])
                nc.vector.tensor_copy(vT[b][:, 132 * i + 65:132 * i + 129],
                                      pt[:, 64:128])
            if i == DT // 2 - 1 or i == DT - 1:
                hh = i // (DT // 2)
                c0 = hh * c.VWH
                for b in range(NTK):
                    off = b * 128 * c.VWH
                    nc.sync.dma_start(
                        vsh[hh][off:off + 128 * c.VWH].rearrange(
                            "(p t) -> p t", p=128),
                        vT[b][:, c0:c0 + c.VWH])
                nc.gpsimd.collective_compute(
                    "AllGather", mybir.AluOpType.bypass,
                    replica_groups=groups,
                    ins=[vsh[hh].opt()], outs=[vall[hh].opt()])

        # --- q section ---
        for i, pm in proj_section(layer, 0, hts):
            rope(pm, qp[i], i)

        # --- attention: head pair i uses K feature tile i, V col block i ---
        for i in range(DT):
            kh = i // (DT // 2)
            bidx = i % (DT // 2)
            # one DMA for all four shards' K feature tile i: [128, 4*T]
            kg = _t(kpool, [128, 4 * T], BF16, "kg")
            ksrc = kall[kh].rearrange("(s b p t) -> p s b t", s=4, p=128, t=T)
            nc.sync.dma_start(kg[:].rearrange("p (s t) -> p s t", t=T),
                              ksrc[:, :, bidx, :])
            # one DMA for all four shards' V column block i: [128, 4*NTK*132]
            ic = bidx * 132
            vg = _t(vgpool, [128, 4 * NTK * 132], BF16, "vg")
            vsrc = vall[kh].rearrange("(s b p t) -> p s b t", s=4, p=128,
                                      t=c.VWH)
            nc.sync.dma_start(
                vg[:].rearrange("p (s b t) -> p s b t", s=4, t=132),
                vsrc[:, :, :, ic:ic + 132])

            aop = _t(spool2, [128, T], BF16, "aop")
            blocks = [(ck, s) for ck in range(NTK) for s in range(4)]
            NB = len(blocks)
            for hh in range(2):
                r0 = hh * 64
                o_ps = _t(ps_o, [128, T], F32, "oaug")
                elist = [None] * NB

                def emit_qk(n):
                    ck, s = blocks[n]
                    q0 = ck * 128
                    st = _t(ps_s, [128, T], F32, "st")
                    nc.tensor.matmul(st[:, q0:T],
                                     kg[r0:r0 + 64,
                                        s * T + q0:s * T + q0 + 128],
                                     qp[i][r0:r0 + 64, q0:T],
                                     start=True, stop=False,
                                     skip_group_check=True)
                    # causal mask for the diagonal block, accumulated on PE:
                    # st[:, q0:q0+128] += I^T @ mask[s]
                    nc.tensor.matmul(st[:, q0:q0 + 128], identb_t[:],
                                     mask_t[s][:], start=False, stop=True,
                                     skip_group_check=True)
                    e = _t(epool, [128, T], BF16, "e")
                    nc.scalar.activation(e[:, q0:T], st[:, q0:T], AF.Exp,
                                         scale=scale)
                    elist[n] = e

                def emit_av(n):
                    ck, s = blocks[n]
                    q0 = ck * 128
                    e = elist[n]
                    # [v_half | ones]: rows 0:64 = o, row 64 = den
                    c0 = s * NTK * 132 + 132 * ck + 65 * hh
                    nc.tensor.matmul(
                        o_ps[0:65, q0:T],
                        vg[:, c0:c0 + 65],
                        e[:, q0:T],
                        start=(n == 0), stop=(n == NB - 1),
                        skip_group_check=True)

                LEAD = 4
                for n in range(min(LEAD, NB)):
                    emit_qk(n)
                for n in range(NB):
                    if n + LEAD < NB:
                        emit_qk(n + LEAD)
                    emit_av(n)
                den = _t(spool2, [1, T], F32, "den")
                nc.vector.tensor_copy(den[:], o_ps[64:65, :])
                denb = _t(spool2, [128, T], F32, "denb")
                nc.gpsimd.partition_broadcast(denb[:], den[:])
                rb128 = _t(spool2, [128, T], F32, "recb")
                nc.vector.reciprocal_approx_fast(rb128[:], denb[:])
                nc.vector.tensor_mul(aop[r0:r0 + 64, :], o_ps[0:64, :],
                                     rb128[r0:r0 + 64, :])
            # out-projection contribution of this head pair, into residual
            wo0 = load_w(io["owT"][layer], i * 128, 0, cols=min(WCOL, D))
            wo1 = load_w(io["owT"][layer], i * 128, WCOL, tagp="wb") \
                if D > WCOL else None
            for oc in range(DT):
                pm = _t(ps, [128, T], F32, "mm")
                col = oc * 128
                wt = wo0 if col < WCOL else wo1
                cc = col % WCOL
                nc.tensor.matmul(pm[:], wt[:, cc:cc + 128], aop[:],
                                 start=True, stop=True)
                nc.vector.tensor_add(xt[oc][:], xt[oc][:], pm[:])

        # ================= FFN (SwiGLU), quarter-fused =================
        hts = rmsnorm(BF16)
        for q0f in range(0, FT, FQ):
            f1 = min(q0f + FQ, FT)
            nf = f1 - q0f
            w1 = [load_w(io["w1T"][layer], k * 128, q0f * 128,
                         cols=min(WCOL, nf * 128)) for k in range(DT)]
            g = []
            for f in range(q0f, f1):
                pm = _t(ps, [128, T], F32, "mm")
                for k in range(DT):
                    cc = (f - q0f) * 128
                    nc.tensor.matmul(pm[:], w1[k][:, cc:cc + 128], hts[k][:],
                                     start=(k == 0), stop=(k == DT - 1))
                sg = _t(spool2, [128, T], F32, "sg")
                nc.scalar.activation(sg[:], pm[:], AF.Sigmoid)
                gt = _t(gpool, [128, T], F32, f"g{f - q0f}")
                nc.vector.tensor_mul(gt[:], sg[:], pm[:])
                g.append(gt)
            w3 = [load_w(io["w3T"][layer], k * 128, q0f * 128,
                         cols=min(WCOL, nf * 128), tagp="wb")
                  for k in range(DT)]
            gu = []
            for f in range(q0f, f1):
                pm = _t(ps, [128, T], F32, "mm")
                for k in range(DT):
                    cc = (f - q0f) * 128
                    nc.tensor.matmul(pm[:], w3[k][:, cc:cc + 128], hts[k][:],
                                     start=(k == 0), stop=(k == DT - 1))
                gut = _t(gupool, [128, T], BF16, f"gu{f - q0f}")
                nc.vector.tensor_mul(gut[:], g[f - q0f][:], pm[:])
                gu.append(gut)
            w2 = []
            for f in range(q0f, f1):
                wt0 = load_w(io["w2T"][layer], f * 128, 0, cols=min(WCOL, D))
                wt1 = load_w(io["w2T"][layer], f * 128, WCOL, tagp="wb") \
                    if D > WCOL else None
                w2.append((wt0, wt1))
            for oc in range(DT):
                pm = _t(ps, [128, T], F32, "mm")
                col = oc * 128
                for f in range(nf):
                    wt = w2[f][0] if col < WCOL else w2[f][1]
                    cc = col % WCOL
                    nc.tensor.matmul(pm[:], wt[:, cc:cc + 128], gu[f][:],
                                     start=(f == 0), stop=(f == nf - 1))
                nc.vector.tensor_add(xt[oc][:], xt[oc][:], pm[:])

    # ================= final norm + LM head =================
    hts = rmsnorm(F32)
    VB = c.LM_OC * 128
    assert V % VB == 0
    for vb in range(V // VB):
        we = [load_w(io["embT"], k * 128, vb * VB, cols=min(WCOL, VB), dt=F32)
              for k in range(DT)]
        we2 = None
        if VB > WCOL:
            we2 = [_t(wpool, [128, WCOL], F32, f"wb{k % 8}")
                   for k in range(DT)]
            for k in range(DT):
                nc.sync.dma_start(
                    _r(we2[k][:, :VB - WCOL]),
                    _r(io["embT"][k * 128:(k + 1) * 128,
                                  vb * VB + WCOL:(vb + 1) * VB]))
        for o in range(c.LM_OC):
            pm = _t(ps, [128, T], F32, "mm")
            col = o * 128
            for k in range(DT):
                wt = we[k] if col < WCOL else we2[k]
                cc = col % WCOL
                nc.tensor.matmul(pm[:], _r(wt[:, cc:cc + 128]), _r(hts[k][:]),
                                 start=(k == 0), stop=(k == DT - 1))
            sb = _t(spool2, [128, T], F32, "lmo")
            nc.scalar.copy(sb[:], pm[:])
            nc.sync.dma_start(
                io["logits"][vb * VB + col: vb * VB + col + 128, :], sb[:])

    for p in reversed(ctx_pools):
        p.__exit__(None, None, None)


# ============================================================
# host side
# ============================================================

def host_prep(cfg, inputs, n_cores=8):
    c = Ctx(cfg)
    ids = np.asarray(inputs["input_ids"])
    emb = np.asarray(inputs["emb"], np.float32)
    anw = np.asarray(inputs["attn_norm_w"], np.float32)
    fnw = np.asarray(inputs["ffn_norm_w"], np.float32)
    lnw = np.asarray(inputs["final_norm_w"], np.float32)
    bf = ml_dtypes.bfloat16

    qkvT = np.ascontiguousarray(
        (np.transpose(np.asarray(inputs["qkv_w"], np.float32), (0, 2, 1))
         * anw[:, :, None]).astype(bf))
    owT = np.ascontiguousarray(
        np.transpose(np.asarray(inputs["out_w"], np.float32),
                     (0, 2, 1)).astype(bf))
    w1T = np.ascontiguousarray(
        (np.transpose(np.asarray(inputs["w1"], np.float32), (0, 2, 1))
         * fnw[:, :, None]).astype(bf))
    w3T = np.ascontiguousarray(
        (np.transpose(np.asarray(inputs["w3"], np.float32), (0, 2, 1))
         * fnw[:, :, None]).astype(bf))
    w2T = np.ascontiguousarray(
        np.transpose(np.asarray(inputs["w2"], np.float32),
                     (0, 2, 1)).astype(bf))
    embT = np.ascontiguousarray(emb.T * lnw[:, None])

    hd = c.HD
    inv = 1.0 / (10000.0 ** (np.arange(0, hd, 2, dtype=np.float32) / hd))

    P = np.zeros((128, 128), np.float32)
    for head in range(2):
        b = head * 64
        for m in range(32):
            P[b + m + 32, b + m] = -1.0   # rot[j] = -q[j+32], j < 32
            P[b + m, b + m + 32] = 1.0    # rot[j] =  q[j-32], j >= 32
    k_idx = np.arange(128)[:, None]
    j_idx = np.arange(128)[None, :]
    tri_incl = np.where(k_idx <= j_idx, 0.0, NEG).astype(np.float32)
    tri_strict = np.where(k_idx < j_idx, 0.0, NEG).astype(np.float32)

    in_maps = []
    for core in range(n_cores):
        b, p = core // 4, core % 4
        tok = np.asarray(ids[b, p::4], np.int64)
        x0 = np.ascontiguousarray(emb[tok].T)
        pos = np.arange(p, c.S, 4, dtype=np.float32)
        fr = pos[:, None] * inv[None, :]
        ang = np.concatenate([fr, fr], axis=1)          # [T, hd]
        cosq = np.ascontiguousarray(np.tile(np.cos(ang).T, (2, 1)).astype(bf))
        sinq = np.ascontiguousarray(np.tile(np.sin(ang).T, (2, 1)).astype(bf))
        maskA = np.ascontiguousarray(
            np.stack([tri_incl if s <= p else tri_strict
                      for s in range(4)]).astype(bf))
        in_maps.append(dict(
            x0=x0, qkvT=qkvT, owT=owT, w1T=w1T, w3T=w3T, w2T=w2T, embT=embT,
            cosq=cosq, sinq=sinq, ropeP=P.astype(bf), maskA=maskA))
    return in_maps


def assemble(cfg, results):
    c = Ctx(cfg)
    out = np.empty((c.B, c.S, c.V), np.float32)
    for core in range(len(results)):
        b, p = core // 4, core % 4
        out[b, p::4, :] = results[core]["logits"].T
    return out


_NC_CACHE = {}


def kernel(**inputs):
    cfg = CFG_FULL
    if "full" not in _NC_CACHE:
        _NC_CACHE["full"] = build_nc(cfg)
    nc = _NC_CACHE["full"]
    in_maps = host_prep(cfg, inputs)
    res = run_bass_kernel_spmd(nc, in_maps, list(range(8)))
    return assemble(cfg, res.results)


# revision 36
# speedup vs baseline: 1.2983x; 1.0357x over previous
"""MicroTransformer forward pass on 8 trn2 NeuronCores.

Sharding: DP2 (batch) x CP4 (strided context parallel).
Core c = (b, p), b = c // 4, p = c % 4, owns tokens at global positions
p, p+4, p+8, ... of batch b  (T = S/4 tokens per core).

Per layer: k-projection first (K shard dumped + AllGathered in two
feature halves), then v (transposed + ones-augmented, gathered in two
column halves), then q — so the gathers run under projection compute.
Attention per head pair: one wide QK^T matmul per (shard, key-chunk)
into transposed scores [tk, tq]; exp feeds one wide accumulating AV
matmul per (shard, key-chunk).  Out-proj and SwiGLU FFN are
token-local.  LM head: own tokens x full vocab in f32r.  Norm weights
fold into the following weight matrix on the host; weights arrive
pre-transposed and in bf16 so the contraction dim lies on SBUF
partitions.

On-chip layout: activations are feature-major [feat, tok]; matmul
outputs [out_feat, tok] feed the next matmul's moving operand directly.
Partition-dim reductions (RMSNorm sum, softmax denom) use ones-vector /
ones-column matmuls on the PE.
"""

import numpy as np

try:
    import concourse.bass as bass  # noqa: F401
except ImportError:
    import sys

    sys.path.insert(0, "/opt/trn_rl_repo")
    import concourse.bass as bass  # noqa: F401

import ml_dtypes

import concourse.bacc as bacc
import concourse.mybir as mybir
import concourse.tile as tile
import concourse.bass_utils as _bu
from concourse.bass_utils import run_bass_kernel_spmd
from concourse.masks import make_identity

# (LDW optimization was tried here and rejected: walrus hard-errors with
# "InstLdweights is not compatible with LDW optimization" on transposes.)

F32 = mybir.dt.float32
F32R = mybir.dt.float32r
BF16 = mybir.dt.bfloat16
AF = mybir.ActivationFunctionType

NEG = -1e30

CFG_FULL = dict(V=32000, D=1024, L=8, F=4096, S=2048, H=16, HD=64, LM_OC=5)


class Ctx:
    def __init__(self, cfg):
        self.__dict__.update(cfg)
        self.B = 2
        self.T = self.S // 4        # tokens per core
        self.DT = self.D // 128     # 128-row feature tiles of x/h
        self.FT = self.F // 128
        self.NTK = self.T // 128    # tk chunks per shard (= tq blocks)
        assert self.T % 128 == 0 and self.D % 128 == 0 and self.HD == 64
        self.VW = (self.H // 2) * 132  # per pair: ones|v_even|ones|v_odd|pp
        self.VWH = self.VW // 2
        self.KH = (self.D // 2) * self.T       # K elems per feature half
        self.VH = self.T * self.VWH            # V elems per column half


def _t(pool, shape, dtype, tag):
    return pool.tile(shape, dtype, tag=tag, name=tag)


def _r(ap):
    return ap.bitcast(F32R)


def build_nc(cfg, n_cores=8):
    c = Ctx(cfg)
    nc = bacc.Bacc("TRN2", target_bir_lowering=False, debug=False,
                   num_devices=n_cores)
    if n_cores == 8:
        groups = [[0, 1, 2, 3], [4, 5, 6, 7]]
    else:
        groups = [list(range(n_cores))]

    D, T, L, F, V = c.D, c.T, c.L, c.F, c.V

    io = {}
    def inp(name, shape, dt=F32):
        io[name] = nc.dram_tensor(name, shape, dt, kind="ExternalInput").ap()
    inp("x0", [D, T])
    inp("qkvT", [L, D, 3 * D], BF16)
    inp("owT", [L, D, D], BF16)
    inp("w1T", [L, D, F], BF16)
    inp("w3T", [L, D, F], BF16)
    inp("w2T", [L, F, D], BF16)
    inp("embT", [D, V], BF16)
    inp("cosq", [128, T], BF16)
    inp("sinq", [128, T], BF16)
    inp("ropeP", [128, 128], BF16)
    inp("maskA", [4, 128, 128], BF16)
    io["logits"] = nc.dram_tensor("logits", [V, T], F32,
                                  kind="ExternalOutput").ap()

    with tile.TileContext(nc) as tc:
        _emit(tc, c, groups, io)
    nc.compile()
    return nc


def _emit(tc, c, groups, io):
    nc = tc.nc
    D, T, L, F, V, H = c.D, c.T, c.L, c.F, c.V, c.H
    DT, FT, NTK, VW = c.DT, c.FT, c.NTK, c.VW
    FQ = 4 if FT % 4 == 0 else FT
    scale = c.HD ** -0.5
    WCOL = 512

    ctx_pools = []
    def pool(**kw):
        p = tc.tile_pool(**kw)
        v = p.__enter__()
        ctx_pools.append(p)
        return v

    perst = pool(name="perst", bufs=1)
    wpool = pool(name="wpool", bufs=3)      # streamed weight tiles [128, 512]
    apool = pool(name="apool", bufs=1)      # per-layer activations (by tag)
    spool1 = pool(name="spool1", bufs=1)    # norm staging
    spool2 = pool(name="spool2", bufs=2)    # rotating staging tiles
    kpool = pool(name="kpool", bufs=2)      # gathered K tiles, per shard
    vgpool = pool(name="vgpool", bufs=2)    # gathered V tiles, per shard
    epool = pool(name="epool", bufs=6)      # exp tiles
    gpool = pool(name="gpool", bufs=1)      # gate tiles
    gupool = pool(name="gupool", bufs=1)    # gate*up tiles
    ps = pool(name="ps", bufs=3, space="PSUM")
    ps_s = pool(name="ps_s", bufs=3, space="PSUM")
    ps_o = pool(name="ps_o", bufs=2, space="PSUM")
    dram = pool(name="dram", bufs=2, space="DRAM")

    # ---------- persistent tiles ----------
    xt = [_t(perst, [128, T], F32, f"x{i}") for i in range(DT)]
    cos_t = _t(perst, [128, T], BF16, "cos")
    sin_t = _t(perst, [128, T], BF16, "sin")
    ropeP_t = _t(perst, [128, 128], BF16, "ropeP")
    ident_t = _t(perst, [128, 128], F32, "ident")
    identb_t = _t(perst, [128, 128], BF16, "identb")
    mask_t = [_t(perst, [128, 128], BF16, f"mask{s}") for s in range(4)]
    ones_t = _t(perst, [128, 1], F32, "ones")
    eps_t = _t(perst, [1, 1], F32, "eps")
    nc.gpsimd.memset(eps_t[:], 1e-6)

    nc.sync.dma_start(cos_t[:], io["cosq"][:])
    nc.sync.dma_start(sin_t[:], io["sinq"][:])
    nc.sync.dma_start(ropeP_t[:], io["ropeP"][:])
    for s in range(4):
        nc.sync.dma_start(mask_t[s][:], io["maskA"][s])
    ones_raw = _t(perst, [128, 1], F32, "ones_raw")
    nc.gpsimd.memset(ones_raw[:], 1.0)
    nc.vector.tensor_copy(_r(ones_t[:]), ones_raw[:])
    make_identity(nc, ident_t[:])
    nc.vector.tensor_copy(identb_t[:], ident_t[:])
    for i in range(DT):
        nc.sync.dma_start(xt[i][:], io["x0"][i * 128:(i + 1) * 128, :])

    def load_w(dram_ap, r0, c0, rows=128, cols=WCOL, dt=BF16, tagp="w"):
        t = _t(wpool, [128, WCOL], dt, f"{tagp}{(r0 // 128) % 8}")
        if dt == F32:
            nc.sync.dma_start(_r(t[:rows, :cols]),
                              _r(dram_ap[r0:r0 + rows, c0:c0 + cols]))
        else:
            nc.sync.dma_start(t[:rows, :cols],
                              dram_ap[r0:r0 + rows, c0:c0 + cols])
        return t

    def rmsnorm(out_dt):
        """h = x * rsqrt(mean(x^2) + eps). Returns h tiles (feature-major)."""
        ssum = _t(ps, [128, T], F32, "mm")
        for i in range(DT):
            sqt = _t(spool1, [128, T], F32, "nsq")
            nc.vector.tensor_mul(_r(sqt[:]), xt[i][:], xt[i][:])
            nc.tensor.matmul(ssum[0:1, :], _r(ones_t[:]), _r(sqt[:]),
                             start=(i == 0), stop=(i == DT - 1))
        srt = _t(spool1, [1, T], F32, "nsrt")
        nc.scalar.activation(srt[:], ssum[0:1, :], AF.Sqrt,
                             bias=eps_t[:], scale=1.0 / D)
        sb = _t(spool1, [128, T], F32, "nsb")
        nc.gpsimd.partition_broadcast(sb[:], srt[:])
        rb = _t(spool1, [128, T], F32, "nrb")
        nc.vector.reciprocal_approx_fast(rb[:], sb[:])
        hts = []
        for i in range(DT):
            h = _t(apool, [128, T], out_dt, f"h{i}")
            dst = _r(h[:]) if out_dt == F32 else h[:]
            nc.vector.tensor_mul(dst, xt[i][:], rb[:])
            hts.append(h)
        return hts

    def proj_section(layer, which, hts):
        """Compute one D-section of qkv: returns list of PSUM tiles consumed
        by caller one at a time (generator to keep PSUM pressure low)."""
        wsec = []
        for k in range(DT):
            wt0 = load_w(io["qkvT"][layer], k * 128, which * D,
                         cols=min(WCOL, D))
            wt1 = load_w(io["qkvT"][layer], k * 128, which * D + WCOL,
                         tagp="wb") if D > WCOL else None
            wsec.append((wt0, wt1))
        for i in range(DT):
            pm = _t(ps, [128, T], F32, "mm")
            col = i * 128
            for k in range(DT):
                wt = wsec[k][0] if col < WCOL else wsec[k][1]
                cc = col % WCOL
                nc.tensor.matmul(pm[:], wt[:, cc:cc + 128], hts[k][:],
                                 start=(k == 0), stop=(k == DT - 1))
            yield i, pm

    def rope(pm, out, i):
        """out = pm*cos + rotate_half(pm)*sin   (bf16 out)."""
        sb = _t(spool2, [128, T], BF16, "rsb")
        nc.vector.tensor_copy(sb[:], pm[:])
        rot = _t(ps, [128, T], F32, "mm")
        nc.tensor.matmul(rot[:], ropeP_t[:], sb[:], start=True, stop=True)
        t1 = _t(spool2, [128, T], BF16, "rope1")
        nc.vector.tensor_mul(t1[:], sb[:], cos_t[:])
        t2 = _t(spool2, [128, T], BF16, "rope2")
        nc.vector.tensor_mul(t2[:], rot[:], sin_t[:])
        nc.vector.tensor_add(out[:], t1[:], t2[:])

    for layer in range(L):
        # ================= attention =================
        hts = rmsnorm(BF16)

        qp = [_t(apool, [128, T], BF16, f"qp{i}") for i in range(DT)]
        vT = [_t(apool, [128, VW], BF16, f"vT{b}") for b in range(NTK)]
        for b in range(NTK):
            nc.gpsimd.memset(vT[b][:], 1.0)

        ksh = [dram.tile([c.KH], BF16, tag=f"ksh{h}", name=f"ksh{h}")
               for h in range(2)]
        kall = [dram.tile([4 * c.KH], BF16, tag=f"kall{h}", name=f"kall{h}")
                for h in range(2)]
        vsh = [dram.tile([c.VH], BF16, tag=f"vsh{h}", name=f"vsh{h}")
               for h in range(2)]
        vall = [dram.tile([4 * c.VH], BF16, tag=f"vall{h}", name=f"vall{h}")
                for h in range(2)]

        # --- k section: project + RoPE, dump, gather per feature half ---
        for i, pm in proj_section(layer, 1, hts):
            kb = _t(spool2, [128, T], BF16, "kb")
            rope(pm, kb, i)
            hh = i // (DT // 2)
            off = (i % (DT // 2)) * 128 * T
            nc.sync.dma_start(
                ksh[hh][off:off + 128 * T].rearrange("(p t) -> p t", p=128),
                kb[:])
            if i % (DT // 2) == DT // 2 - 1:
                nc.gpsimd.collective_compute(
                    "AllGather", mybir.AluOpType.bypass,
                    replica_groups=groups,
                    ins=[ksh[hh].opt()], outs=[kall[hh].opt()])

        # --- v section: project, transpose into augmented layout, dump ---
        for i, pm in proj_section(layer, 2, hts):
            vsb = _t(spool2, [128, T], BF16, "vsb")
            nc.vector.tensor_copy(vsb[:], pm[:])
            for b in range(NTK):
                pt = _t(ps_s, [128, 128], BF16, "st")
                nc.tensor.transpose(pt[:], vsb[:, b * 128:(b + 1) * 128],
                                    identb_t[:])
                nc.vector.tensor_copy(vT[b][:, 132 * i:132 * i + 64],
                                      pt[:, 0:64])
                nc.vector.tensor_copy(vT[b][:, 132 * i + 65:132 * i + 129],
                                      pt[:, 64:128])
            if i == DT // 2 - 1 or i == DT - 1:
                hh = i // (DT // 2)
                c0 = hh * c.VWH
                for b in range(NTK):
                    off = b * 128 * c.VWH
                    nc.sync.dma_start(
                        vsh[hh][off:off + 128 * c.VWH].rearrange(
                            "(p t) -> p t", p=128),
                        vT[b][:, c0:c0 + c.VWH])
                nc.gpsimd.collective_compute(
                    "AllGather", mybir.AluOpType.bypass,
                    replica_groups=groups,
                    ins=[vsh[hh].opt()], outs=[vall[hh].opt()])

        # --- q section ---
        for i, pm in proj_section(layer, 0, hts):
            rope(pm, qp[i], i)

        # --- attention: head pair i uses K feature tile i, V col block i ---
        for i in range(DT):
            kh = i // (DT // 2)
            bidx = i % (DT // 2)
            # one DMA for all four shards' K feature tile i: [128, 4*T]
            kg = _t(kpool, [128, 4 * T], BF16, "kg")
            ksrc = kall[kh].rearrange("(s b p t) -> p s b t", s=4, p=128, t=T)
            nc.sync.dma_start(kg[:].rearrange("p (s t) -> p s t", t=T),
                              ksrc[:, :, bidx, :])
            # one DMA for all four shards' V column block i: [128, 4*NTK*132]
            ic = bidx * 132
            vg = _t(vgpool, [128, 4 * NTK * 132], BF16, "vg")
            vsrc = vall[kh].rearrange("(s b p t) -> p s b t", s=4, p=128,
                                      t=c.VWH)
            nc.sync.dma_start(
                vg[:].rearrange("p (s b t) -> p s b t", s=4, t=132),
                vsrc[:, :, :, ic:ic + 132])

            aop = _t(spool2, [128, T], BF16, "aop")
            blocks = [(ck, s) for ck in range(NTK) for s in range(4)]
            NB = len(blocks)
            for hh in range(2):
                r0 = hh * 64
                o_ps = _t(ps_o, [128, T], F32, "oaug")
                elist = [None] * NB

                def emit_qk(n):
                    ck, s = blocks[n]
                    q0 = ck * 128
                    st = _t(ps_s, [128, T], F32, "st")
                    nc.tensor.matmul(st[:, q0:T],
                                     kg[r0:r0 + 64,
                                        s * T + q0:s * T + q0 + 128],
                                     qp[i][r0:r0 + 64, q0:T],
                                     start=True, stop=False,
                                     skip_group_check=True)
                    # causal mask for the diagonal block, accumulated on PE:
                    # st[:, q0:q0+128] += I^T @ mask[s]
                    nc.tensor.matmul(st[:, q0:q0 + 128], identb_t[:],
                                     mask_t[s][:], start=False, stop=True,
                                     skip_group_check=True)
                    e = _t(epool, [128, T], BF16, "e")
                    nc.scalar.activation(e[:, q0:T], st[:, q0:T], AF.Exp,
                                         scale=scale)
                    elist[n] = e

                def emit_av(n):
                    ck, s = blocks[n]
                    q0 = ck * 128
                    e = elist[n]
                    # [v_half | ones]: rows 0:64 = o, row 64 = den
                    c0 = s * NTK * 132 + 132 * ck + 65 * hh
                    nc.tensor.matmul(
                        o_ps[0:65, q0:T],
                        vg[:, c0:c0 + 65],
                        e[:, q0:T],
                        start=(n == 0), stop=(n == NB - 1),
                        skip_group_check=True)

                LEAD = 3
                for n in range(min(LEAD, NB)):
                    emit_qk(n)
                for n in range(NB):
                    if n + LEAD < NB:
                        emit_qk(n + LEAD)
                    emit_av(n)
                den = _t(spool2, [1, T], F32, "den")
                nc.vector.tensor_copy(den[:], o_ps[64:65, :])
                denb = _t(spool2, [128, T], F32, "denb")
                nc.gpsimd.partition_broadcast(denb[:], den[:])
                rb128 = _t(spool2, [128, T], F32, "recb")
                nc.vector.reciprocal_approx_fast(rb128[:], denb[:])
                nc.vector.tensor_mul(aop[r0:r0 + 64, :], o_ps[0:64, :],
                                     rb128[r0:r0 + 64, :])
            # out-projection contribution of this head pair, into residual
            wo0 = load_w(io["owT"][layer], i * 128, 0, cols=min(WCOL, D))
            wo1 = load_w(io["owT"][layer], i * 128, WCOL, tagp="wb") \
                if D > WCOL else None
            for oc in range(DT):
                pm = _t(ps, [128, T], F32, "mm")
                col = oc * 128
                wt = wo0 if col < WCOL else wo1
                cc = col % WCOL
                nc.tensor.matmul(pm[:], wt[:, cc:cc + 128], aop[:],
                                 start=True, stop=True)
                nc.vector.tensor_add(xt[oc][:], xt[oc][:], pm[:])

        # ================= FFN (SwiGLU), quarter-fused =================
        hts = rmsnorm(BF16)
        for q0f in range(0, FT, FQ):
            f1 = min(q0f + FQ, FT)
            nf = f1 - q0f
            w1 = [load_w(io["w1T"][layer], k * 128, q0f * 128,
                         cols=min(WCOL, nf * 128)) for k in range(DT)]
            g = []
            for f in range(q0f, f1):
                pm = _t(ps, [128, T], F32, "mm")
                for k in range(DT):
                    cc = (f - q0f) * 128
                    nc.tensor.matmul(pm[:], w1[k][:, cc:cc + 128], hts[k][:],
                                     start=(k == 0), stop=(k == DT - 1))
                sg = _t(spool2, [128, T], F32, "sg")
                nc.scalar.activation(sg[:], pm[:], AF.Sigmoid)
                gt = _t(gpool, [128, T], BF16, f"g{f - q0f}")
                nc.vector.tensor_mul(gt[:], sg[:], pm[:])
                g.append(gt)
            w3 = [load_w(io["w3T"][layer], k * 128, q0f * 128,
                         cols=min(WCOL, nf * 128), tagp="wb")
                  for k in range(DT)]
            gu = []
            for f in range(q0f, f1):
                pm = _t(ps, [128, T], F32, "mm")
                for k in range(DT):
                    cc = (f - q0f) * 128
                    nc.tensor.matmul(pm[:], w3[k][:, cc:cc + 128], hts[k][:],
                                     start=(k == 0), stop=(k == DT - 1))
                gut = _t(gupool, [128, T], BF16, f"gu{f - q0f}")
                nc.vector.tensor_mul(gut[:], g[f - q0f][:], pm[:])
                gu.append(gut)
            w2 = []
            for f in range(q0f, f1):
                wt0 = load_w(io["w2T"][layer], f * 128, 0, cols=min(WCOL, D))
                wt1 = load_w(io["w2T"][layer], f * 128, WCOL, tagp="wb") \
                    if D > WCOL else None
                w2.append((wt0, wt1))
            for oc in range(DT):
                pm = _t(ps, [128, T], F32, "mm")
                col = oc * 128
                for f in range(nf):
                    wt = w2[f][0] if col < WCOL else w2[f][1]
                    cc = col % WCOL
                    nc.tensor.matmul(pm[:], wt[:, cc:cc + 128], gu[f][:],
                                     start=(f == 0), stop=(f == nf - 1))
                nc.vector.tensor_add(xt[oc][:], xt[oc][:], pm[:])

    # ================= final norm + LM head =================
    hts = rmsnorm(BF16)
    VB = c.LM_OC * 128
    assert V % VB == 0
    for vb in range(V // VB):
        we = [load_w(io["embT"], k * 128, vb * VB, cols=min(WCOL, VB))
              for k in range(DT)]
        we2 = None
        if VB > WCOL:
            we2 = [_t(wpool, [128, WCOL], BF16, f"wb{k % 8}")
                   for k in range(DT)]
            for k in range(DT):
                nc.sync.dma_start(
                    we2[k][:, :VB - WCOL],
                    io["embT"][k * 128:(k + 1) * 128,
                               vb * VB + WCOL:(vb + 1) * VB])
        for o in range(c.LM_OC):
            pm = _t(ps, [128, T], F32, "mm")
            col = o * 128
            for k in range(DT):
                wt = we[k] if col < WCOL else we2[k]
                cc = col % WCOL
                nc.tensor.matmul(pm[:], wt[:, cc:cc + 128], hts[k][:],
                                 start=(k == 0), stop=(k == DT - 1))
            sb = _t(spool2, [128, T], F32, "lmo")
            nc.scalar.copy(sb[:], pm[:])
            nc.sync.dma_start(
                io["logits"][vb * VB + col: vb * VB + col + 128, :], sb[:])

    for p in reversed(ctx_pools):
        p.__exit__(None, None, None)


# ============================================================
# host side
# ============================================================

def host_prep(cfg, inputs, n_cores=8):
    c = Ctx(cfg)
    ids = np.asarray(inputs["input_ids"])
    emb = np.asarray(inputs["emb"], np.float32)
    anw = np.asarray(inputs["attn_norm_w"], np.float32)
    fnw = np.asarray(inputs["ffn_norm_w"], np.float32)
    lnw = np.asarray(inputs["final_norm_w"], np.float32)
    bf = ml_dtypes.bfloat16

    qkvT = np.ascontiguousarray(
        (np.transpose(np.asarray(inputs["qkv_w"], np.float32), (0, 2, 1))
         * anw[:, :, None]).astype(bf))
    owT = np.ascontiguousarray(
        np.transpose(np.asarray(inputs["out_w"], np.float32),
                     (0, 2, 1)).astype(bf))
    w1T = np.ascontiguousarray(
        (np.transpose(np.asarray(inputs["w1"], np.float32), (0, 2, 1))
         * fnw[:, :, None]).astype(bf))
    w3T = np.ascontiguousarray(
        (np.transpose(np.asarray(inputs["w3"], np.float32), (0, 2, 1))
         * fnw[:, :, None]).astype(bf))
    w2T = np.ascontiguousarray(
        np.transpose(np.asarray(inputs["w2"], np.float32),
                     (0, 2, 1)).astype(bf))
    embT = np.ascontiguousarray((emb.T * lnw[:, None]).astype(bf))

    hd = c.HD
    inv = 1.0 / (10000.0 ** (np.arange(0, hd, 2, dtype=np.float32) / hd))

    P = np.zeros((128, 128), np.float32)
    for head in range(2):
        b = head * 64
        for m in range(32):
            P[b + m + 32, b + m] = -1.0   # rot[j] = -q[j+32], j < 32
            P[b + m, b + m + 32] = 1.0    # rot[j] =  q[j-32], j >= 32
    k_idx = np.arange(128)[:, None]
    j_idx = np.arange(128)[None, :]
    tri_incl = np.where(k_idx <= j_idx, 0.0, NEG).astype(np.float32)
    tri_strict = np.where(k_idx < j_idx, 0.0, NEG).astype(np.float32)

    in_maps = []
    for core in range(n_cores):
        b, p = core // 4, core % 4
        tok = np.asarray(ids[b, p::4], np.int64)
        x0 = np.ascontiguousarray(emb[tok].T)
        pos = np.arange(p, c.S, 4, dtype=np.float32)
        fr = pos[:, None] * inv[None, :]
        ang = np.concatenate([fr, fr], axis=1)          # [T, hd]
        cosq = np.ascontiguousarray(np.tile(np.cos(ang).T, (2, 1)).astype(bf))
        sinq = np.ascontiguousarray(np.tile(np.sin(ang).T, (2, 1)).astype(bf))
        maskA = np.ascontiguousarray(
            np.stack([tri_incl if s <= p else tri_strict
                      for s in range(4)]).astype(bf))
        in_maps.append(dict(
            x0=x0, qkvT=qkvT, owT=owT, w1T=w1T, w3T=w3T, w2T=w2T, embT=embT,
            cosq=cosq, sinq=sinq, ropeP=P.astype(bf), maskA=maskA))
    return in_maps


def assemble(cfg, results):
    c = Ctx(cfg)
    out = np.empty((c.B, c.S, c.V), np.float32)
    for core in range(len(results)):
        b, p = core // 4, core % 4
        out[b, p::4, :] = results[core]["logits"].T
    return out


_NC_CACHE = {}


def kernel(**inputs):
    cfg = CFG_FULL
    if "full" not in _NC_CACHE:
        _NC_CACHE["full"] = build_nc(cfg)
    nc = _NC_CACHE["full"]
    in_maps = host_prep(cfg, inputs)
    res = run_bass_kernel_spmd(nc, in_maps, list(range(8)))
    return assemble(cfg, res.results)


# revision 38
# speedup vs baseline: 1.3231x; 1.0191x over previous
"""MicroTransformer forward pass on 8 trn2 NeuronCores.

Sharding: DP2 (batch) x CP4 (strided context parallel).
Core c = (b, p), b = c // 4, p = c % 4, owns tokens at global positions
p, p+4, p+8, ... of batch b  (T = S/4 tokens per core).

Per layer: k-projection first (K shard dumped + AllGathered in two
feature halves), then v (transposed + ones-augmented, gathered in two
column halves), then q — so the gathers run under projection compute.
Attention per head pair: one wide QK^T matmul per (shard, key-chunk)
into transposed scores [tk, tq]; exp feeds one wide accumulating AV
matmul per (shard, key-chunk).  Out-proj and SwiGLU FFN are
token-local.  LM head: own tokens x full vocab in f32r.  Norm weights
fold into the following weight matrix on the host; weights arrive
pre-transposed and in bf16 so the contraction dim lies on SBUF
partitions.

On-chip layout: activations are feature-major [feat, tok]; matmul
outputs [out_feat, tok] feed the next matmul's moving operand directly.
Partition-dim reductions (RMSNorm sum, softmax denom) use ones-vector /
ones-column matmuls on the PE.
"""

import numpy as np

try:
    import concourse.bass as bass  # noqa: F401
except ImportError:
    import sys

    sys.path.insert(0, "/opt/trn_rl_repo")
    import concourse.bass as bass  # noqa: F401

import ml_dtypes

import concourse.bacc as bacc
import concourse.mybir as mybir
import concourse.tile as tile
import concourse.bass_utils as _bu
from concourse.bass_utils import run_bass_kernel_spmd
from concourse.masks import make_identity

# (LDW optimization was tried here and rejected: walrus hard-errors with
# "InstLdweights is not compatible with LDW optimization" on transposes.)

F32 = mybir.dt.float32
F32R = mybir.dt.float32r
BF16 = mybir.dt.bfloat16
AF = mybir.ActivationFunctionType

NEG = -1e30

CFG_FULL = dict(V=32000, D=1024, L=8, F=4096, S=2048, H=16, HD=64, LM_OC=5)


class Ctx:
    def __init__(self, cfg):
        self.__dict__.update(cfg)
        self.B = 2
        self.T = self.S // 4        # tokens per core
        self.DT = self.D // 128     # 128-row feature tiles of x/h
        self.FT = self.F // 128
        self.NTK = self.T // 128    # tk chunks per shard (= tq blocks)
        assert self.T % 128 == 0 and self.D % 128 == 0 and self.HD == 64
        self.VW = (self.H // 2) * 132  # per pair: ones|v_even|ones|v_odd|pp
        self.VWH = self.VW // 2
        self.KH = (self.D // 2) * self.T       # K elems per feature half
        self.VH = self.T * self.VWH            # V elems per column half


def _t(pool, shape, dtype, tag):
    return pool.tile(shape, dtype, tag=tag, name=tag)


def _r(ap):
    return ap.bitcast(F32R)


def build_nc(cfg, n_cores=8):
    c = Ctx(cfg)
    nc = bacc.Bacc("TRN2", target_bir_lowering=False, debug=False,
                   num_devices=n_cores)
    if n_cores == 8:
        groups = [[0, 1, 2, 3], [4, 5, 6, 7]]
    else:
        groups = [list(range(n_cores))]

    D, T, L, F, V = c.D, c.T, c.L, c.F, c.V

    io = {}
    def inp(name, shape, dt=F32):
        io[name] = nc.dram_tensor(name, shape, dt, kind="ExternalInput").ap()
    inp("x0", [D, T])
    inp("qkvT", [L, D, 3 * D], BF16)
    inp("owT", [L, D, D], BF16)
    inp("w1T", [L, D, F], BF16)
    inp("w3T", [L, D, F], BF16)
    inp("w2T", [L, F, D], BF16)
    inp("embT", [D, V], BF16)
    inp("cosq", [128, T], BF16)
    inp("sinq", [128, T], BF16)
    inp("ropeP", [128, 128], BF16)
    inp("maskA", [4, 128, 128], BF16)
    io["logits"] = nc.dram_tensor("logits", [V, T], F32,
                                  kind="ExternalOutput").ap()

    with tile.TileContext(nc) as tc:
        _emit(tc, c, groups, io)
    nc.compile()
    return nc


def _emit(tc, c, groups, io):
    nc = tc.nc
    D, T, L, F, V, H = c.D, c.T, c.L, c.F, c.V, c.H
    DT, FT, NTK, VW = c.DT, c.FT, c.NTK, c.VW
    FQ = 4 if FT % 4 == 0 else FT
    scale = c.HD ** -0.5
    WCOL = 512

    ctx_pools = []
    def pool(**kw):
        p = tc.tile_pool(**kw)
        v = p.__enter__()
        ctx_pools.append(p)
        return v

    perst = pool(name="perst", bufs=1)
    wpool = pool(name="wpool", bufs=3)      # streamed weight tiles [128, 512]
    apool = pool(name="apool", bufs=1)      # per-layer activations (by tag)
    spool1 = pool(name="spool1", bufs=1)    # norm staging
    spool2 = pool(name="spool2", bufs=2)    # rotating staging tiles
    kpool = pool(name="kpool", bufs=2)      # gathered K tiles, per shard
    vgpool = pool(name="vgpool", bufs=2)    # gathered V tiles, per shard
    epool = pool(name="epool", bufs=6)      # exp tiles
    gpool = pool(name="gpool", bufs=1)      # gate tiles
    gupool = pool(name="gupool", bufs=1)    # gate*up tiles
    ps = pool(name="ps", bufs=3, space="PSUM")
    ps_s = pool(name="ps_s", bufs=3, space="PSUM")
    ps_o = pool(name="ps_o", bufs=2, space="PSUM")
    dram = pool(name="dram", bufs=2, space="DRAM")

    # ---------- persistent tiles ----------
    xt = [_t(perst, [128, T], F32, f"x{i}") for i in range(DT)]
    cos_t = _t(perst, [128, T], BF16, "cos")
    sin_t = _t(perst, [128, T], BF16, "sin")
    ropeP_t = _t(perst, [128, 128], BF16, "ropeP")
    ident_t = _t(perst, [128, 128], F32, "ident")
    identb_t = _t(perst, [128, 128], BF16, "identb")
    mask_t = [_t(perst, [128, 128], BF16, f"mask{s}") for s in range(4)]
    ones_t = _t(perst, [128, 1], F32, "ones")
    eps_t = _t(perst, [1, 1], F32, "eps")
    nc.gpsimd.memset(eps_t[:], 1e-6)

    nc.sync.dma_start(cos_t[:], io["cosq"][:])
    nc.sync.dma_start(sin_t[:], io["sinq"][:])
    nc.sync.dma_start(ropeP_t[:], io["ropeP"][:])
    for s in range(4):
        nc.sync.dma_start(mask_t[s][:], io["maskA"][s])
    ones_raw = _t(perst, [128, 1], F32, "ones_raw")
    nc.gpsimd.memset(ones_raw[:], 1.0)
    nc.vector.tensor_copy(_r(ones_t[:]), ones_raw[:])
    make_identity(nc, ident_t[:])
    nc.vector.tensor_copy(identb_t[:], ident_t[:])
    for i in range(DT):
        nc.sync.dma_start(xt[i][:], io["x0"][i * 128:(i + 1) * 128, :])

    def load_w(dram_ap, r0, c0, rows=128, cols=WCOL, dt=BF16, tagp="w"):
        t = _t(wpool, [128, WCOL], dt, f"{tagp}{(r0 // 128) % 8}")
        if dt == F32:
            nc.sync.dma_start(_r(t[:rows, :cols]),
                              _r(dram_ap[r0:r0 + rows, c0:c0 + cols]))
        else:
            nc.sync.dma_start(t[:rows, :cols],
                              dram_ap[r0:r0 + rows, c0:c0 + cols])
        return t

    def rmsnorm(out_dt):
        """h = x * rsqrt(mean(x^2) + eps). Returns h tiles (feature-major)."""
        ssum = _t(ps, [128, T], F32, "mm")
        for i in range(DT):
            sqt = _t(spool1, [128, T], F32, "nsq")
            nc.vector.tensor_mul(_r(sqt[:]), xt[i][:], xt[i][:])
            nc.tensor.matmul(ssum[0:1, :], _r(ones_t[:]), _r(sqt[:]),
                             start=(i == 0), stop=(i == DT - 1))
        srt = _t(spool1, [1, T], F32, "nsrt")
        nc.scalar.activation(srt[:], ssum[0:1, :], AF.Sqrt,
                             bias=eps_t[:], scale=1.0 / D)
        sb = _t(spool1, [128, T], F32, "nsb")
        nc.gpsimd.partition_broadcast(sb[:], srt[:])
        rb = _t(spool1, [128, T], F32, "nrb")
        nc.vector.reciprocal_approx_fast(rb[:], sb[:])
        hts = []
        for i in range(DT):
            h = _t(apool, [128, T], out_dt, f"h{i}")
            dst = _r(h[:]) if out_dt == F32 else h[:]
            nc.vector.tensor_mul(dst, xt[i][:], rb[:])
            hts.append(h)
        return hts

    def proj_section(layer, which, hts, i_range=None):
        """Compute one D-section of qkv: yields (i, psum) one at a time."""
        wsec = []
        for k in range(DT):
            wt0 = load_w(io["qkvT"][layer], k * 128, which * D,
                         cols=min(WCOL, D))
            wt1 = load_w(io["qkvT"][layer], k * 128, which * D + WCOL,
                         tagp="wb") if D > WCOL else None
            wsec.append((wt0, wt1))
        for i in (range(DT) if i_range is None else i_range):
            pm = _t(ps, [128, T], F32, "mm")
            col = i * 128
            for k in range(DT):
                wt = wsec[k][0] if col < WCOL else wsec[k][1]
                cc = col % WCOL
                nc.tensor.matmul(pm[:], wt[:, cc:cc + 128], hts[k][:],
                                 start=(k == 0), stop=(k == DT - 1))
            yield i, pm

    def rope(pm, out, i):
        """out = pm*cos + rotate_half(pm)*sin   (bf16 out)."""
        sb = _t(spool2, [128, T], BF16, "rsb")
        nc.vector.tensor_copy(sb[:], pm[:])
        rot = _t(ps, [128, T], F32, "mm")
        nc.tensor.matmul(rot[:], ropeP_t[:], sb[:], start=True, stop=True)
        t1 = _t(spool2, [128, T], BF16, "rope1")
        nc.vector.tensor_mul(t1[:], sb[:], cos_t[:])
        t2 = _t(spool2, [128, T], BF16, "rope2")
        nc.vector.tensor_mul(t2[:], rot[:], sin_t[:])
        nc.vector.tensor_add(out[:], t1[:], t2[:])

    for layer in range(L):
        # ================= attention =================
        hts = rmsnorm(BF16)

        qp = [_t(apool, [128, T], BF16, f"qp{i}") for i in range(DT)]
        vT = [_t(apool, [128, VW], BF16, f"vT{b}") for b in range(NTK)]
        for b in range(NTK):
            nc.gpsimd.memset(vT[b][:], 1.0)

        ksh = [dram.tile([c.KH], BF16, tag=f"ksh{h}", name=f"ksh{h}")
               for h in range(2)]
        kall = [dram.tile([4 * c.KH], BF16, tag=f"kall{h}", name=f"kall{h}")
                for h in range(2)]
        vsh = [dram.tile([c.VH], BF16, tag=f"vsh{h}", name=f"vsh{h}")
               for h in range(2)]
        vall = [dram.tile([4 * c.VH], BF16, tag=f"vall{h}", name=f"vall{h}")
                for h in range(2)]

        # --- k/v sections per half: K-AG(h) then V-AG(h) so the collective
        # order matches the attention i-loop's consumption order ---
        DH = DT // 2
        for half in range(2):
            ir = range(half * DH, (half + 1) * DH)
            for i, pm in proj_section(layer, 1, hts, ir):
                kb = _t(spool2, [128, T], BF16, "kb")
                rope(pm, kb, i)
                off = (i % DH) * 128 * T
                nc.sync.dma_start(
                    ksh[half][off:off + 128 * T].rearrange(
                        "(p t) -> p t", p=128),
                    kb[:])
            nc.gpsimd.collective_compute(
                "AllGather", mybir.AluOpType.bypass,
                replica_groups=groups,
                ins=[ksh[half].opt()], outs=[kall[half].opt()])

            for i, pm in proj_section(layer, 2, hts, ir):
                vsb = _t(spool2, [128, T], BF16, "vsb")
                nc.vector.tensor_copy(vsb[:], pm[:])
                for b in range(NTK):
                    pt = _t(ps_s, [128, 128], BF16, "st")
                    nc.tensor.transpose(pt[:], vsb[:, b * 128:(b + 1) * 128],
                                        identb_t[:])
                    nc.vector.tensor_copy(vT[b][:, 132 * i:132 * i + 64],
                                          pt[:, 0:64])
                    nc.vector.tensor_copy(vT[b][:, 132 * i + 65:132 * i + 129],
                                          pt[:, 64:128])
            c0 = half * c.VWH
            for b in range(NTK):
                off = b * 128 * c.VWH
                nc.sync.dma_start(
                    vsh[half][off:off + 128 * c.VWH].rearrange(
                        "(p t) -> p t", p=128),
                    vT[b][:, c0:c0 + c.VWH])
            nc.gpsimd.collective_compute(
                "AllGather", mybir.AluOpType.bypass,
                replica_groups=groups,
                ins=[vsh[half].opt()], outs=[vall[half].opt()])

        # --- q section ---
        for i, pm in proj_section(layer, 0, hts):
            rope(pm, qp[i], i)

        # --- attention: head pair i uses K feature tile i, V col block i ---
        for i in range(DT):
            kh = i // (DT // 2)
            bidx = i % (DT // 2)
            # one DMA for all four shards' K feature tile i: [128, 4*T]
            kg = _t(kpool, [128, 4 * T], BF16, "kg")
            ksrc = kall[kh].rearrange("(s b p t) -> p s b t", s=4, p=128, t=T)
            nc.sync.dma_start(kg[:].rearrange("p (s t) -> p s t", t=T),
                              ksrc[:, :, bidx, :])
            # one DMA for all four shards' V column block i: [128, 4*NTK*132]
            ic = bidx * 132
            vg = _t(vgpool, [128, 4 * NTK * 132], BF16, "vg")
            vsrc = vall[kh].rearrange("(s b p t) -> p s b t", s=4, p=128,
                                      t=c.VWH)
            nc.sync.dma_start(
                vg[:].rearrange("p (s b t) -> p s b t", s=4, t=132),
                vsrc[:, :, :, ic:ic + 132])

            aop = _t(spool2, [128, T], BF16, "aop")
            blocks = [(ck, s) for ck in range(NTK) for s in range(4)]
            NB = len(blocks)
            for hh in range(2):
                r0 = hh * 64
                o_ps = _t(ps_o, [128, T], F32, "oaug")
                elist = [None] * NB

                def emit_qk(n):
                    ck, s = blocks[n]
                    q0 = ck * 128
                    st = _t(ps_s, [128, T], F32, "st")
                    nc.tensor.matmul(st[:, q0:T],
                                     kg[r0:r0 + 64,
                                        s * T + q0:s * T + q0 + 128],
                                     qp[i][r0:r0 + 64, q0:T],
                                     start=True, stop=False,
                                     skip_group_check=True)
                    # causal mask for the diagonal block, accumulated on PE:
                    # st[:, q0:q0+128] += I^T @ mask[s]
                    nc.tensor.matmul(st[:, q0:q0 + 128], identb_t[:],
                                     mask_t[s][:], start=False, stop=True,
                                     skip_group_check=True)
                    e = _t(epool, [128, T], BF16, "e")
                    nc.scalar.activation(e[:, q0:T], st[:, q0:T], AF.Exp,
                                         scale=scale)
                    elist[n] = e

                def emit_av(n):
                    ck, s = blocks[n]
                    q0 = ck * 128
                    e = elist[n]
                    # [v_half | ones]: rows 0:64 = o, row 64 = den
                    c0 = s * NTK * 132 + 132 * ck + 65 * hh
                    nc.tensor.matmul(
                        o_ps[0:65, q0:T],
                        vg[:, c0:c0 + 65],
                        e[:, q0:T],
                        start=(n == 0), stop=(n == NB - 1),
                        skip_group_check=True)

                LEAD = 3
                for n in range(min(LEAD, NB)):
                    emit_qk(n)
                for n in range(NB):
                    if n + LEAD < NB:
                        emit_qk(n + LEAD)
                    emit_av(n)
                den = _t(spool2, [1, T], F32, "den")
                nc.vector.tensor_copy(den[:], o_ps[64:65, :])
                denb = _t(spool2, [128, T], F32, "denb")
                nc.gpsimd.partition_broadcast(denb[:], den[:])
                rb128 = _t(spool2, [128, T], F32, "recb")
                nc.vector.reciprocal_approx_fast(rb128[:], denb[:])
                nc.vector.tensor_mul(aop[r0:r0 + 64, :], o_ps[0:64, :],
                                     rb128[r0:r0 + 64, :])
            # out-projection contribution of this head pair, into residual
            wo0 = load_w(io["owT"][layer], i * 128, 0, cols=min(WCOL, D))
            wo1 = load_w(io["owT"][layer], i * 128, WCOL, tagp="wb") \
                if D > WCOL else None
            for oc in range(DT):
                pm = _t(ps, [128, T], F32, "mm")
                col = oc * 128
                wt = wo0 if col < WCOL else wo1
                cc = col % WCOL
                nc.tensor.matmul(pm[:], wt[:, cc:cc + 128], aop[:],
                                 start=True, stop=True)
                nc.vector.tensor_add(xt[oc][:], xt[oc][:], pm[:])

        # ================= FFN (SwiGLU), quarter-fused =================
        hts = rmsnorm(BF16)
        for q0f in range(0, FT, FQ):
            f1 = min(q0f + FQ, FT)
            nf = f1 - q0f
            w1 = [load_w(io["w1T"][layer], k * 128, q0f * 128,
                         cols=min(WCOL, nf * 128)) for k in range(DT)]
            g = []
            for f in range(q0f, f1):
                pm = _t(ps, [128, T], F32, "mm")
                for k in range(DT):
                    cc = (f - q0f) * 128
                    nc.tensor.matmul(pm[:], w1[k][:, cc:cc + 128], hts[k][:],
                                     start=(k == 0), stop=(k == DT - 1))
                sg = _t(spool2, [128, T], F32, "sg")
                nc.scalar.activation(sg[:], pm[:], AF.Sigmoid)
                gt = _t(gpool, [128, T], BF16, f"g{f - q0f}")
                nc.vector.tensor_mul(gt[:], sg[:], pm[:])
                g.append(gt)
            w3 = [load_w(io["w3T"][layer], k * 128, q0f * 128,
                         cols=min(WCOL, nf * 128), tagp="wb")
                  for k in range(DT)]
            gu = []
            for f in range(q0f, f1):
                pm = _t(ps, [128, T], F32, "mm")
                for k in range(DT):
                    cc = (f - q0f) * 128
                    nc.tensor.matmul(pm[:], w3[k][:, cc:cc + 128], hts[k][:],
                                     start=(k == 0), stop=(k == DT - 1))
                gut = _t(gupool, [128, T], BF16, f"gu{f - q0f}")
                nc.vector.tensor_mul(gut[:], g[f - q0f][:], pm[:])
                gu.append(gut)
            w2 = []
            for f in range(q0f, f1):
                wt0 = load_w(io["w2T"][layer], f * 128, 0, cols=min(WCOL, D))
                wt1 = load_w(io["w2T"][layer], f * 128, WCOL, tagp="wb") \
                    if D > WCOL else None
                w2.append((wt0, wt1))
            for oc in range(DT):
                pm = _t(ps, [128, T], F32, "mm")
                col = oc * 128
                for f in range(nf):
                    wt = w2[f][0] if col < WCOL else w2[f][1]
                    cc = col % WCOL
                    nc.tensor.matmul(pm[:], wt[:, cc:cc + 128], gu[f][:],
                                     start=(f == 0), stop=(f == nf - 1))
                nc.vector.tensor_add(xt[oc][:], xt[oc][:], pm[:])

    # ================= final norm + LM head =================
    hts = rmsnorm(BF16)
    VB = c.LM_OC * 128
    assert V % VB == 0
    for vb in range(V // VB):
        we = [load_w(io["embT"], k * 128, vb * VB, cols=min(WCOL, VB))
              for k in range(DT)]
        we2 = None
        if VB > WCOL:
            we2 = [_t(wpool, [128, WCOL], BF16, f"wb{k % 8}")
                   for k in range(DT)]
            for k in range(DT):
                nc.sync.dma_start(
                    we2[k][:, :VB - WCOL],
                    io["embT"][k * 128:(k + 1) * 128,
                               vb * VB + WCOL:(vb + 1) * VB])
        for o in range(c.LM_OC):
            pm = _t(ps, [128, T], F32, "mm")
            col = o * 128
            for k in range(DT):
                wt = we[k] if col < WCOL else we2[k]
                cc = col % WCOL
                nc.tensor.matmul(pm[:], wt[:, cc:cc + 128], hts[k][:],
                                 start=(k == 0), stop=(k == DT - 1))
            sb = _t(spool2, [128, T], F32, "lmo")
            nc.scalar.copy(sb[:], pm[:])
            nc.sync.dma_start(
                io["logits"][vb * VB + col: vb * VB + col + 128, :], sb[:])

    for p in reversed(ctx_pools):
        p.__exit__(None, None, None)


# ============================================================
# host side
# ============================================================

def host_prep(cfg, inputs, n_cores=8):
    c = Ctx(cfg)
    ids = np.asarray(inputs["input_ids"])
    emb = np.asarray(inputs["emb"], np.float32)
    anw = np.asarray(inputs["attn_norm_w"], np.float32)
    fnw = np.asarray(inputs["ffn_norm_w"], np.float32)
    lnw = np.asarray(inputs["final_norm_w"], np.float32)
    bf = ml_dtypes.bfloat16

    qkvT = np.ascontiguousarray(
        (np.transpose(np.asarray(inputs["qkv_w"], np.float32), (0, 2, 1))
         * anw[:, :, None]).astype(bf))
    owT = np.ascontiguousarray(
        np.transpose(np.asarray(inputs["out_w"], np.float32),
                     (0, 2, 1)).astype(bf))
    w1T = np.ascontiguousarray(
        (np.transpose(np.asarray(inputs["w1"], np.float32), (0, 2, 1))
         * fnw[:, :, None]).astype(bf))
    w3T = np.ascontiguousarray(
        (np.transpose(np.asarray(inputs["w3"], np.float32), (0, 2, 1))
         * fnw[:, :, None]).astype(bf))
    w2T = np.ascontiguousarray(
        np.transpose(np.asarray(inputs["w2"], np.float32),
                     (0, 2, 1)).astype(bf))
    embT = np.ascontiguousarray((emb.T * lnw[:, None]).astype(bf))

    hd = c.HD
    inv = 1.0 / (10000.0 ** (np.arange(0, hd, 2, dtype=np.float32) / hd))

    P = np.zeros((128, 128), np.float32)
    for head in range(2):
        b = head * 64
        for m in range(32):
            P[b + m + 32, b + m] = -1.0   # rot[j] = -q[j+32], j < 32
            P[b + m, b + m + 32] = 1.0    # rot[j] =  q[j-32], j >= 32
    k_idx = np.arange(128)[:, None]
    j_idx = np.arange(128)[None, :]
    tri_incl = np.where(k_idx <= j_idx, 0.0, NEG).astype(np.float32)
    tri_strict = np.where(k_idx < j_idx, 0.0, NEG).astype(np.float32)

    in_maps = []
    for core in range(n_cores):
        b, p = core // 4, core % 4
        tok = np.asarray(ids[b, p::4], np.int64)
        x0 = np.ascontiguousarray(emb[tok].T)
        pos = np.arange(p, c.S, 4, dtype=np.float32)
        fr = pos[:, None] * inv[None, :]
        ang = np.concatenate([fr, fr], axis=1)          # [T, hd]
        cosq = np.ascontiguousarray(np.tile(np.cos(ang).T, (2, 1)).astype(bf))
        sinq = np.ascontiguousarray(np.tile(np.sin(ang).T, (2, 1)).astype(bf))
        maskA = np.ascontiguousarray(
            np.stack([tri_incl if s <= p else tri_strict
                      for s in range(4)]).astype(bf))
        in_maps.append(dict(
            x0=x0, qkvT=qkvT, owT=owT, w1T=w1T, w3T=w3T, w2T=w2T, embT=embT,
            cosq=cosq, sinq=sinq, ropeP=P.astype(bf), maskA=maskA))
    return in_maps


def assemble(cfg, results):
    c = Ctx(cfg)
    out = np.empty((c.B, c.S, c.V), np.float32)
    for core in range(len(results)):
        b, p = core // 4, core % 4
        out[b, p::4, :] = results[core]["logits"].T
    return out


_NC_CACHE = {}


def kernel(**inputs):
    cfg = CFG_FULL
    if "full" not in _NC_CACHE:
        _NC_CACHE["full"] = build_nc(cfg)
    nc = _NC_CACHE["full"]
    in_maps = host_prep(cfg, inputs)
    res = run_bass_kernel_spmd(nc, in_maps, list(range(8)))
    return assemble(cfg, res.results)
